# revision 2
# baseline (speedup 1.0000x reference)
"""NoisyNet dense layer on 8 TRN2 NeuronCores — baseline structure +
hidden weight loads + fp8 DoubleRow on half the noise contraction.

out[b,u] = x @ W_mu + eps_out * ((x*eps_in) @ W_sigma) + bias_mu + bias_sigma*eps_out

Same transposed layout and two-phase, stall-free DMA schedule as the
135.7us baseline ([D,B]/[U,B] on-device, data-parallel over batch), with
two measured upgrades:
  - every matmul is emitted as explicit LDWEIGHTS + a non-self-loading
    MATMUL (ins.ldweights=False): the PE pulls the loads ahead into the
    background weight buffer, cutting ~50ns/MM (265 -> ~216).
  - the noise matmul runs d<1024 in fp8e4 DoubleRow (K=256/instruction
    at the full 2x MAC rate) and d>=1024 in bf16. Full-fp8 would be
    rel-err 2.3e-2 (>2e-2 budget); this 50/50 split measures 1.65e-2.
    eps_in is prescaled by 4 and W_sigma by 1024 (powers of two; z*4
    fits fp8e4's +-240 range); the 1/4096 descale folds into the
    phase-2 ACT bias-add (scale operand), which costs nothing.
"""

import numpy as np
import ml_dtypes

import concourse.bacc as bacc
import concourse.mybir as mybir
import concourse.tile as tile
from concourse.bass_utils import run_bass_kernel_spmd

N_CORES = 8
B, D, U = 4096, 2048, 2048
BL = B // N_CORES          # 512 batch rows per core
P = 128                    # partitions
KT = D // P                # 16 contraction tiles
K8 = KT // 2               # 8 k-tiles in the fp8 half
UT = U // P                # 16 output tiles
KC = 4                     # k-tiles per activation DMA chunk
NCH = KT // KC             # 4 chunks
BF16 = mybir.dt.bfloat16
FP32 = mybir.dt.float32
FP8 = mybir.dt.float8e4

S_Z = 4.0
S_W = 1024.0
DESCALE = 1.0 / (S_Z * S_W)

_NBF = ml_dtypes.bfloat16
_NF8 = ml_dtypes.float8_e4m3

_cached = None


def _build():
    nc = bacc.Bacc("TRN2", target_bir_lowering=False, debug=False)

    # activations laid out [P, KT, BL]: partition p holds d = k*128+p
    xT = nc.declare_dram_parameter("xT", [P, KT, BL], BF16, isOutput=False)
    eiT = nc.declare_dram_parameter("eiT", [P, KT, BL], BF16, isOutput=False)
    eoT = nc.declare_dram_parameter("eoT", [P, UT, BL], BF16, isOutput=False)
    wmu = nc.declare_dram_parameter("wmu", [UT, P, KT * P], BF16, isOutput=False)
    wg8 = nc.declare_dram_parameter("wg8", [UT, P, K8 * P], FP8, isOutput=False)
    wgb = nc.declare_dram_parameter("wgb", [UT, P, K8 * P], BF16, isOutput=False)
    bmu = nc.declare_dram_parameter("bmu", [P, UT], FP32, isOutput=False)
    bsg = nc.declare_dram_parameter("bsg", [P, UT], FP32, isOutput=False)
    outT = nc.declare_dram_parameter("outT", [UT, P, BL], FP32, isOutput=True)

    DR = mybir.MatmulPerfMode.DoubleRow
    IDENT = mybir.ActivationFunctionType.Identity

    with tile.TileContext(nc) as tc:
        with (
            tc.tile_pool(name="acts", bufs=1) as acts,
            tc.tile_pool(name="w", bufs=7) as wp,
            tc.tile_pool(name="w8", bufs=4) as wp8,
            tc.tile_pool(name="bias", bufs=1) as bp,
            tc.tile_pool(name="psum", bufs=4, space="PSUM") as pp,
            tc.tile_pool(name="psumn", bufs=3, space="PSUM") as ppn,
            tc.tile_pool(name="mean", bufs=1) as mp,
            tc.tile_pool(name="tmp", bufs=2) as tp,
            tc.tile_pool(name="out", bufs=3) as op,
        ):
            # HAM warm-up so the first real matmuls run at full clock.
            warm_in = bp.tile([P, BL], BF16, tag="warmin")
            nc.gpsimd.memset(warm_in[:], 0.0)
            warm_ps = ppn.tile([P, BL], FP32, tag="psn")
            for _ in range(6):
                nc.tensor.matmul(warm_ps[:], warm_in[:, :P], warm_in[:])

            def mm(ps, stat, mov, start, stop, perf_mode=None):
                nc.tensor.ldweights(stat, perf_mode=perf_mode)
                m = nc.tensor.matmul(ps, stat, mov, start=start, stop=stop,
                                     perf_mode=perf_mode)
                m.ins.ldweights = False

            # Weight stream (sync queue): all W_mu first, then W_sigma
            # (fp8 tile + bf16 tile per u).
            wm_tiles = {}
            ws_tiles = {}

            def fetch_wm(u, split=False):
                t = wp.tile([P, KT * P], BF16, tag="wm")
                q = nc.sync if u % 2 == 0 else nc.gpsimd
                if split:
                    q.dma_start(t[:, :KC * P], wmu[u][:, :KC * P])
                    q.dma_start(t[:, KC * P:], wmu[u][:, KC * P:])
                else:
                    q.dma_start(t[:], wmu[u])
                wm_tiles[u] = t

            def fetch_ws(u):
                t8 = wp8.tile([P, K8, P], FP8, tag="ws8")
                nc.sync.dma_start(t8[:], wg8[u])
                tb = wp.tile([P, K8 * P], BF16, tag="wsb")
                nc.scalar.dma_start(tb[:], wgb[u])
                ws_tiles[u] = (t8, tb)

            # biases first in the gpsimd stream (needed at the first
            # phase-1 drain; the odd-u W_mu stream follows them).
            bmu_t = bp.tile([P, UT], FP32, tag="bmu")
            nc.gpsimd.dma_start(bmu_t[:], bmu[:])
            bsg_t = bp.tile([P, UT], FP32, tag="bsg")
            nc.gpsimd.dma_start(bsg_t[:], bsg[:])

            fetch_wm(0, split=True)

            # Activation stream (scalar queue): x first, then eps_in
            # (z8/zb production), then eps_out.
            x_sb = acts.tile([P, KT, BL], BF16, tag="x")
            z8 = acts.tile([P, K8, BL], FP8, tag="z8")
            zb = acts.tile([P, K8, BL], BF16, tag="zb")
            eo_sb = acts.tile([P, UT, BL], BF16, tag="eo")

            nc.scalar.dma_start(x_sb[:, 0:1, :], xT[:, 0:1, :])
            nc.scalar.dma_start(x_sb[:, 1:KC, :], xT[:, 1:KC, :])
            fetch_wm(1)
            for c in range(1, NCH):
                s = slice(c * KC, (c + 1) * KC)
                nc.scalar.dma_start(x_sb[:, s, :], xT[:, s, :])
            for uu in range(2, 6):
                fetch_wm(uu)
            for c in range(NCH):
                s = slice(c * KC, (c + 1) * KC)
                ei_c = acts.tile([P, KC, BL], BF16, tag=f"ei{c}",
                                 name=f"ei{c}")
                nc.scalar.dma_start(ei_c[:], eiT[:, s, :])
                for kk in range(KC):
                    k = c * KC + kk
                    if k < K8:
                        nc.vector.tensor_mul(z8[:, k, :], x_sb[:, k, :],
                                             ei_c[:, kk, :])
                    else:
                        nc.vector.tensor_mul(zb[:, k - K8, :], x_sb[:, k, :],
                                             ei_c[:, kk, :])

            # ---- Phase 1: mean terms. t_m[u] = W_mu[u].T @ x + bias_mu ----
            t_m = []
            for u in range(UT):
                if u + 6 < UT:
                    fetch_wm(u + 6)
                elif u + 6 == UT:
                    for uu in range(3):
                        fetch_ws(uu)
                wm = wm_tiles.pop(u)
                pm = pp.tile([P, BL], FP32, tag="psm")
                for k in range(KT):
                    mm(pm[:], wm[:, k * P:(k + 1) * P], x_sb[:, k, :],
                       start=(k == 0), stop=(k == KT - 1))
                tm = mp.tile([P, BL], FP32, tag=f"tm{u}")
                nc.scalar.add(tm[:], pm[:], bmu_t[:, u:u + 1])
                t_m.append(tm)

            # eps_out on gpsimd, behind the phase-1 odd-u W_mu fetches.
            for c in range(NCH):
                s = slice(c * KC, (c + 1) * KC)
                nc.gpsimd.dma_start(eo_sb[:, s, :], eoT[:, s, :])

            # ---- Phase 2: noise terms + combine. d<1024 fp8 DR, rest bf16 ----
            for u in range(UT):
                un = u + 3
                if 3 <= un < UT:
                    fetch_ws(un)
                ws8, wsb = ws_tiles.pop(u)
                last = (u == UT - 1)
                halves = (0, BL // 2, BL) if last else (0, BL)
                for h in range(len(halves) - 1):
                    lo, hi = halves[h], halves[h + 1]
                    pn = ppn.tile([P, hi - lo], FP32, tag="psn")
                    for k2 in range(K8 // 2):
                        mm(pn[:], ws8[:, 2 * k2:2 * k2 + 2, :],
                           z8[:, 2 * k2:2 * k2 + 2, lo:hi],
                           start=(k2 == 0), stop=False, perf_mode=DR)
                    for k in range(K8):
                        mm(pn[:], wsb[:, k * P:(k + 1) * P], zb[:, k, lo:hi],
                           start=False, stop=(k == K8 - 1))
                    t_n = tp.tile([P, hi - lo], FP32, tag="tn")
                    nc.scalar.activation(t_n[:], pn[:], IDENT,
                                         bias=bsg_t[:, u:u + 1], scale=DESCALE)
                    pr = tp.tile([P, hi - lo], FP32, tag="pr")
                    nc.vector.tensor_mul(pr[:], t_n[:], eo_sb[:, u, lo:hi])
                    o = op.tile([P, hi - lo], FP32, tag="o")
                    nc.vector.tensor_add(o[:], pr[:], t_m[u][:, lo:hi])
                    nc.gpsimd.dma_start(outT[u][:, lo:hi], o[:])

    nc.compile()
    return nc


def _get_nc():
    global _cached
    if _cached is None:
        _cached = _build()
    return _cached


def kernel(x, weight_mu, weight_sigma, bias_mu, bias_sigma, eps_in, eps_out,
           _trace=False):
    nc = _get_nc()

    # Host-side layout prep (transposes + casts + quantization scaling only).
    def to_pkb(a):  # [B, D] -> per-core [P, KT, BL]
        a = np.ascontiguousarray(a.astype(_NBF))
        return [
            np.ascontiguousarray(
                a[c * BL:(c + 1) * BL].T.reshape(KT, P, BL).transpose(1, 0, 2))
            for c in range(N_CORES)
        ]

    xs = to_pkb(x)
    eis = to_pkb(eps_in * S_Z)
    eos = to_pkb(eps_out)  # same transform, u in place of k

    def w_blocks(w, scale, dt):  # [D', U] -> [UT, P, (D'/128)*P]
        kt = w.shape[0] // P
        wb = (w * scale).astype(dt).reshape(kt, P, UT, P).transpose(2, 1, 0, 3)
        return np.ascontiguousarray(wb.reshape(UT, P, kt * P))

    wmu_h = w_blocks(weight_mu, 1.0, _NBF)
    wg8_h = w_blocks(weight_sigma[:D // 2], S_W, _NF8)
    wgb_h = w_blocks(weight_sigma[D // 2:], S_W, _NBF)
    bmu_h = np.ascontiguousarray(bias_mu.astype(np.float32).reshape(UT, P).T)
    bsg_h = np.ascontiguousarray(bias_sigma.astype(np.float32).reshape(UT, P).T)

    in_maps = [
        {
            "xT": xs[c],
            "eiT": eis[c],
            "eoT": eos[c],
            "wmu": wmu_h,
            "wg8": wg8_h,
            "wgb": wgb_h,
            "bmu": bmu_h,
            "bsg": bsg_h,
        }
        for c in range(N_CORES)
    ]

    res = run_bass_kernel_spmd(nc, in_maps, core_ids=list(range(N_CORES)),
                               trace=_trace)
    kernel.last_result = res

    out = np.empty((B, U), dtype=np.float32)
    for c in range(N_CORES):
        oc = res.results[c]["outT"]  # [UT, P, BL]
        out[c * BL:(c + 1) * BL] = oc.transpose(2, 0, 1).reshape(BL, U)
    return out


# revision 3
# speedup vs baseline: 1.0073x; 1.0073x over previous
"""NoisyNet dense layer on 8 TRN2 NeuronCores — baseline structure +
hidden weight loads + fp8 DoubleRow on half the noise contraction.

out[b,u] = x @ W_mu + eps_out * ((x*eps_in) @ W_sigma) + bias_mu + bias_sigma*eps_out

Same transposed layout and two-phase, stall-free DMA schedule as the
135.7us baseline ([D,B]/[U,B] on-device, data-parallel over batch), with
two measured upgrades:
  - every matmul is emitted as explicit LDWEIGHTS + a non-self-loading
    MATMUL (ins.ldweights=False): the PE pulls the loads ahead into the
    background weight buffer, cutting ~50ns/MM (265 -> ~216).
  - the noise matmul runs d<1024 in fp8e4 DoubleRow (K=256/instruction
    at the full 2x MAC rate) and d>=1024 in bf16. Full-fp8 would be
    rel-err 2.3e-2 (>2e-2 budget); this 50/50 split measures 1.65e-2.
    eps_in is prescaled by 4 and W_sigma by 1024 (powers of two; z*4
    fits fp8e4's +-240 range); the 1/4096 descale folds into the
    phase-2 ACT bias-add (scale operand), which costs nothing.
"""

import numpy as np
import ml_dtypes

import concourse.bacc as bacc
import concourse.mybir as mybir
import concourse.tile as tile
from concourse.bass_utils import run_bass_kernel_spmd

N_CORES = 8
B, D, U = 4096, 2048, 2048
BL = B // N_CORES          # 512 batch rows per core
P = 128                    # partitions
KT = D // P                # 16 contraction tiles
K8 = KT // 2               # 8 k-tiles in the fp8 half
UT = U // P                # 16 output tiles
KC = 4                     # k-tiles per activation DMA chunk
NCH = KT // KC             # 4 chunks
BF16 = mybir.dt.bfloat16
FP32 = mybir.dt.float32
FP8 = mybir.dt.float8e4

S_Z = 4.0
S_W = 1024.0
DESCALE = 1.0 / (S_Z * S_W)

_NBF = ml_dtypes.bfloat16
_NF8 = ml_dtypes.float8_e4m3

_cached = None


def _build():
    nc = bacc.Bacc("TRN2", target_bir_lowering=False, debug=False)

    # activations laid out [P, KT, BL]: partition p holds d = k*128+p
    xT = nc.declare_dram_parameter("xT", [P, KT, BL], BF16, isOutput=False)
    eiT = nc.declare_dram_parameter("eiT", [P, KT, BL], BF16, isOutput=False)
    eoT = nc.declare_dram_parameter("eoT", [P, UT, BL], BF16, isOutput=False)
    wmu = nc.declare_dram_parameter("wmu", [UT, P, KT * P], BF16, isOutput=False)
    wg8 = nc.declare_dram_parameter("wg8", [UT, P, K8 * P], FP8, isOutput=False)
    wgb = nc.declare_dram_parameter("wgb", [UT, P, K8 * P], BF16, isOutput=False)
    bmu = nc.declare_dram_parameter("bmu", [P, UT], FP32, isOutput=False)
    bsg = nc.declare_dram_parameter("bsg", [P, UT], FP32, isOutput=False)
    outT = nc.declare_dram_parameter("outT", [UT, P, BL], FP32, isOutput=True)

    DR = mybir.MatmulPerfMode.DoubleRow
    IDENT = mybir.ActivationFunctionType.Identity

    with tile.TileContext(nc) as tc:
        with (
            tc.tile_pool(name="acts", bufs=1) as acts,
            tc.tile_pool(name="w", bufs=7) as wp,
            tc.tile_pool(name="w8", bufs=4) as wp8,
            tc.tile_pool(name="bias", bufs=1) as bp,
            tc.tile_pool(name="psum", bufs=4, space="PSUM") as pp,
            tc.tile_pool(name="psumn", bufs=3, space="PSUM") as ppn,
            tc.tile_pool(name="mean", bufs=1) as mp,
            tc.tile_pool(name="tmp", bufs=2) as tp,
            tc.tile_pool(name="out", bufs=3) as op,
        ):
            # HAM warm-up so the first real matmuls run at full clock.
            warm_in = bp.tile([P, BL], BF16, tag="warmin")
            nc.gpsimd.memset(warm_in[:], 0.0)
            warm_ps = ppn.tile([P, BL], FP32, tag="psn")
            for _ in range(6):
                nc.tensor.matmul(warm_ps[:], warm_in[:, :P], warm_in[:])

            def mm(ps, stat, mov, start, stop, perf_mode=None):
                nc.tensor.ldweights(stat, perf_mode=perf_mode)
                m = nc.tensor.matmul(ps, stat, mov, start=start, stop=stop,
                                     perf_mode=perf_mode)
                m.ins.ldweights = False

            # Weight stream (sync queue): all W_mu first, then W_sigma
            # (fp8 tile + bf16 tile per u).
            wm_tiles = {}
            ws_tiles = {}

            def fetch_wm(u, split=False):
                t = wp.tile([P, KT * P], BF16, tag="wm")
                q = nc.sync if u % 2 == 0 else nc.gpsimd
                if split:
                    q.dma_start(t[:, :KC * P], wmu[u][:, :KC * P])
                    q.dma_start(t[:, KC * P:], wmu[u][:, KC * P:])
                else:
                    q.dma_start(t[:], wmu[u])
                wm_tiles[u] = t

            def fetch_ws(u):
                t8 = wp8.tile([P, K8, P], FP8, tag="ws8")
                nc.sync.dma_start(t8[:], wg8[u])
                tb = wp.tile([P, K8 * P], BF16, tag="wsb")
                nc.scalar.dma_start(tb[:], wgb[u])
                ws_tiles[u] = (t8, tb)

            # biases first in the gpsimd stream (needed at the first
            # phase-1 drain; the odd-u W_mu stream follows them).
            bmu_t = bp.tile([P, UT], FP32, tag="bmu")
            nc.gpsimd.dma_start(bmu_t[:], bmu[:])
            bsg_t = bp.tile([P, UT], FP32, tag="bsg")
            nc.gpsimd.dma_start(bsg_t[:], bsg[:])

            fetch_wm(0, split=True)

            # Activation stream (scalar queue): x first, then eps_in
            # (z8/zb production), then eps_out.
            x_sb = acts.tile([P, KT, BL], BF16, tag="x")
            z8 = acts.tile([P, K8, BL], FP8, tag="z8")
            zb = acts.tile([P, K8, BL], BF16, tag="zb")
            eo_sb = acts.tile([P, UT, BL], BF16, tag="eo")

            nc.scalar.dma_start(x_sb[:, 0:1, :], xT[:, 0:1, :])
            nc.scalar.dma_start(x_sb[:, 1:KC, :], xT[:, 1:KC, :])
            fetch_wm(1)
            for c in range(1, NCH):
                s = slice(c * KC, (c + 1) * KC)
                nc.scalar.dma_start(x_sb[:, s, :], xT[:, s, :])
            for uu in range(2, 6):
                fetch_wm(uu)
            for c in range(NCH):
                s = slice(c * KC, (c + 1) * KC)
                ei_c = acts.tile([P, KC, BL], BF16, tag=f"ei{c}",
                                 name=f"ei{c}")
                nc.scalar.dma_start(ei_c[:], eiT[:, s, :])
                for kk in range(KC):
                    k = c * KC + kk
                    if k < K8:
                        nc.vector.tensor_mul(z8[:, k, :], x_sb[:, k, :],
                                             ei_c[:, kk, :])
                    else:
                        nc.vector.tensor_mul(zb[:, k - K8, :], x_sb[:, k, :],
                                             ei_c[:, kk, :])

            # ---- Phase 1: mean terms. t_m[u] = W_mu[u].T @ x + bias_mu ----
            t_m = []
            for u in range(UT):
                if u + 6 < UT:
                    fetch_wm(u + 6)
                elif u + 6 == UT:
                    for uu in range(3):
                        fetch_ws(uu)
                wm = wm_tiles.pop(u)
                pm = pp.tile([P, BL], FP32, tag="psm")
                for k in range(KT):
                    mm(pm[:], wm[:, k * P:(k + 1) * P], x_sb[:, k, :],
                       start=(k == 0), stop=(k == KT - 1))
                tm = mp.tile([P, BL], FP32, tag=f"tm{u}")
                nc.scalar.add(tm[:], pm[:], bmu_t[:, u:u + 1])
                t_m.append(tm)

            # eps_out on gpsimd, behind the phase-1 odd-u W_mu fetches.
            for c in range(NCH):
                s = slice(c * KC, (c + 1) * KC)
                nc.gpsimd.dma_start(eo_sb[:, s, :], eoT[:, s, :])

            # ---- Phase 2: noise terms + combine. d<1024 fp8 DR, rest bf16 ----
            for u in range(UT):
                un = u + 3
                if 3 <= un < UT:
                    fetch_ws(un)
                ws8, wsb = ws_tiles.pop(u)
                last = (u >= UT - 2)
                halves = (0, BL // 2, BL) if last else (0, BL)
                for h in range(len(halves) - 1):
                    lo, hi = halves[h], halves[h + 1]
                    pn = ppn.tile([P, hi - lo], FP32, tag="psn")
                    for k2 in range(K8 // 2):
                        mm(pn[:], ws8[:, 2 * k2:2 * k2 + 2, :],
                           z8[:, 2 * k2:2 * k2 + 2, lo:hi],
                           start=(k2 == 0), stop=False, perf_mode=DR)
                    for k in range(K8):
                        mm(pn[:], wsb[:, k * P:(k + 1) * P], zb[:, k, lo:hi],
                           start=False, stop=(k == K8 - 1))
                    t_n = tp.tile([P, hi - lo], FP32, tag="tn")
                    nc.scalar.activation(t_n[:], pn[:], IDENT,
                                         bias=bsg_t[:, u:u + 1], scale=DESCALE)
                    pr = tp.tile([P, hi - lo], FP32, tag="pr")
                    nc.vector.tensor_mul(pr[:], t_n[:], eo_sb[:, u, lo:hi])
                    o = op.tile([P, hi - lo], FP32, tag="o")
                    nc.vector.tensor_add(o[:], pr[:], t_m[u][:, lo:hi])
                    if last:
                        oq = nc.sync if h == 0 else nc.scalar
                    else:
                        oq = nc.gpsimd
                    oq.dma_start(outT[u][:, lo:hi], o[:])

    nc.compile()
    return nc


def _get_nc():
    global _cached
    if _cached is None:
        _cached = _build()
    return _cached


def kernel(x, weight_mu, weight_sigma, bias_mu, bias_sigma, eps_in, eps_out,
           _trace=False):
    nc = _get_nc()

    # Host-side layout prep (transposes + casts + quantization scaling only).
    def to_pkb(a):  # [B, D] -> per-core [P, KT, BL]
        a = np.ascontiguousarray(a.astype(_NBF))
        return [
            np.ascontiguousarray(
                a[c * BL:(c + 1) * BL].T.reshape(KT, P, BL).transpose(1, 0, 2))
            for c in range(N_CORES)
        ]

    xs = to_pkb(x)
    eis = to_pkb(eps_in * S_Z)
    eos = to_pkb(eps_out)  # same transform, u in place of k

    def w_blocks(w, scale, dt):  # [D', U] -> [UT, P, (D'/128)*P]
        kt = w.shape[0] // P
        wb = (w * scale).astype(dt).reshape(kt, P, UT, P).transpose(2, 1, 0, 3)
        return np.ascontiguousarray(wb.reshape(UT, P, kt * P))

    wmu_h = w_blocks(weight_mu, 1.0, _NBF)
    wg8_h = w_blocks(weight_sigma[:D // 2], S_W, _NF8)
    wgb_h = w_blocks(weight_sigma[D // 2:], S_W, _NBF)
    bmu_h = np.ascontiguousarray(bias_mu.astype(np.float32).reshape(UT, P).T)
    bsg_h = np.ascontiguousarray(bias_sigma.astype(np.float32).reshape(UT, P).T)

    in_maps = [
        {
            "xT": xs[c],
            "eiT": eis[c],
            "eoT": eos[c],
            "wmu": wmu_h,
            "wg8": wg8_h,
            "wgb": wgb_h,
            "bmu": bmu_h,
            "bsg": bsg_h,
        }
        for c in range(N_CORES)
    ]

    res = run_bass_kernel_spmd(nc, in_maps, core_ids=list(range(N_CORES)),
                               trace=_trace)
    kernel.last_result = res

    out = np.empty((B, U), dtype=np.float32)
    for c in range(N_CORES):
        oc = res.results[c]["outT"]  # [UT, P, BL]
        out[c * BL:(c + 1) * BL] = oc.transpose(2, 0, 1).reshape(BL, U)
    return out


# revision 4
# speedup vs baseline: 1.0211x; 1.0137x over previous
"""NoisyNet dense layer on 8 TRN2 NeuronCores — baseline structure +
hidden weight loads + fp8 DoubleRow on half the noise contraction.

out[b,u] = x @ W_mu + eps_out * ((x*eps_in) @ W_sigma) + bias_mu + bias_sigma*eps_out

Same transposed layout and two-phase, stall-free DMA schedule as the
135.7us baseline ([D,B]/[U,B] on-device, data-parallel over batch), with
two measured upgrades:
  - every matmul is emitted as explicit LDWEIGHTS + a non-self-loading
    MATMUL (ins.ldweights=False): the PE pulls the loads ahead into the
    background weight buffer, cutting ~50ns/MM (265 -> ~216).
  - the noise matmul runs d<1024 in fp8e4 DoubleRow (K=256/instruction
    at the full 2x MAC rate) and d>=1024 in bf16. Full-fp8 would be
    rel-err 2.3e-2 (>2e-2 budget); this 50/50 split measures 1.65e-2.
    eps_in is prescaled by 4 and W_sigma by 1024 (powers of two; z*4
    fits fp8e4's +-240 range); the 1/4096 descale folds into the
    phase-2 ACT bias-add (scale operand), which costs nothing.
"""

import numpy as np
import ml_dtypes

import concourse.bacc as bacc
import concourse.mybir as mybir
import concourse.tile as tile
from concourse.bass_utils import run_bass_kernel_spmd

N_CORES = 8
B, D, U = 4096, 2048, 2048
BL = B // N_CORES          # 512 batch rows per core
P = 128                    # partitions
KT = D // P                # 16 contraction tiles
K8 = KT // 2               # 8 k-tiles in the fp8 half
UT = U // P                # 16 output tiles
KC = 4                     # k-tiles per activation DMA chunk
NCH = KT // KC             # 4 chunks
BF16 = mybir.dt.bfloat16
FP32 = mybir.dt.float32
FP8 = mybir.dt.float8e4

S_Z = 4.0
S_W = 1024.0
DESCALE = 1.0 / (S_Z * S_W)

_NBF = ml_dtypes.bfloat16
_NF8 = ml_dtypes.float8_e4m3

_cached = None


def _build():
    nc = bacc.Bacc("TRN2", target_bir_lowering=False, debug=False)

    # activations laid out [P, KT, BL]: partition p holds d = k*128+p
    xT = nc.declare_dram_parameter("xT", [P, KT, BL], BF16, isOutput=False)
    eiT = nc.declare_dram_parameter("eiT", [P, KT, BL], BF16, isOutput=False)
    eoT = nc.declare_dram_parameter("eoT", [P, UT, BL], BF16, isOutput=False)
    wmu = nc.declare_dram_parameter("wmu", [UT, P, KT * P], BF16, isOutput=False)
    wg8 = nc.declare_dram_parameter("wg8", [UT, P, K8 * P], FP8, isOutput=False)
    wgb = nc.declare_dram_parameter("wgb", [UT, P, K8 * P], BF16, isOutput=False)
    bmu = nc.declare_dram_parameter("bmu", [P, UT], FP32, isOutput=False)
    bsg = nc.declare_dram_parameter("bsg", [P, UT], FP32, isOutput=False)
    outT = nc.declare_dram_parameter("outT", [UT, P, BL], FP32, isOutput=True)

    DR = mybir.MatmulPerfMode.DoubleRow
    IDENT = mybir.ActivationFunctionType.Identity

    with tile.TileContext(nc) as tc:
        with (
            tc.tile_pool(name="acts", bufs=1) as acts,
            tc.tile_pool(name="w", bufs=7) as wp,
            tc.tile_pool(name="w8", bufs=4) as wp8,
            tc.tile_pool(name="bias", bufs=1) as bp,
            tc.tile_pool(name="psum", bufs=4, space="PSUM") as pp,
            tc.tile_pool(name="psumn", bufs=3, space="PSUM") as ppn,
            tc.tile_pool(name="mean", bufs=1) as mp,
            tc.tile_pool(name="tmp", bufs=2) as tp,
            tc.tile_pool(name="out", bufs=3) as op,
        ):
            # HAM warm-up so the first real matmuls run at full clock.
            warm_in = bp.tile([P, BL], BF16, tag="warmin")
            nc.gpsimd.memset(warm_in[:], 0.0)
            warm_ps = ppn.tile([P, BL], FP32, tag="psn")
            for _ in range(6):
                nc.tensor.matmul(warm_ps[:], warm_in[:, :P], warm_in[:])

            def mm(ps, stat, mov, start, stop, perf_mode=None):
                nc.tensor.ldweights(stat, perf_mode=perf_mode)
                m = nc.tensor.matmul(ps, stat, mov, start=start, stop=stop,
                                     perf_mode=perf_mode)
                m.ins.ldweights = False

            # Weight stream (sync queue): all W_mu first, then W_sigma
            # (fp8 tile + bf16 tile per u).
            wm_tiles = {}
            ws_tiles = {}

            def fetch_wm(u, split=False):
                t = wp.tile([P, KT * P], BF16, tag="wm")
                q = nc.sync if u % 2 == 0 else nc.gpsimd
                if split:
                    q.dma_start(t[:, :KC * P], wmu[u][:, :KC * P])
                    q.dma_start(t[:, KC * P:], wmu[u][:, KC * P:])
                else:
                    q.dma_start(t[:], wmu[u])
                wm_tiles[u] = t

            def fetch_ws(u):
                t8 = wp8.tile([P, K8, P], FP8, tag="ws8")
                nc.sync.dma_start(t8[:], wg8[u])
                tb = wp.tile([P, K8 * P], BF16, tag="wsb")
                nc.scalar.dma_start(tb[:], wgb[u])
                ws_tiles[u] = (t8, tb)

            # biases first in the gpsimd stream (needed at the first
            # phase-1 drain; the odd-u W_mu stream follows them).
            bmu_t = bp.tile([P, UT], FP32, tag="bmu")
            nc.gpsimd.dma_start(bmu_t[:], bmu[:])
            bsg_t = bp.tile([P, UT], FP32, tag="bsg")
            nc.gpsimd.dma_start(bsg_t[:], bsg[:])

            fetch_wm(0, split=True)

            # Activation stream (scalar queue): x first, then eps_in
            # (z8/zb production), then eps_out.
            x_sb = acts.tile([P, KT, BL], BF16, tag="x")
            z8 = acts.tile([P, K8, BL], FP8, tag="z8")
            zb = acts.tile([P, K8, BL], BF16, tag="zb")
            eo_sb = acts.tile([P, UT, BL], BF16, tag="eo")

            nc.scalar.dma_start(x_sb[:, 0:1, :], xT[:, 0:1, :])
            nc.scalar.dma_start(x_sb[:, 1:KC, :], xT[:, 1:KC, :])
            fetch_wm(1)
            for c in range(1, NCH):
                s = slice(c * KC, (c + 1) * KC)
                nc.scalar.dma_start(x_sb[:, s, :], xT[:, s, :])
            for uu in range(2, 6):
                fetch_wm(uu)
            # ---- Phase 1: mean terms. t_m[u] = W_mu[u].T @ x + bias_mu ----
            t_m = []
            for u in range(UT):
                if u + 6 < UT:
                    fetch_wm(u + 6)
                elif u + 6 == UT:
                    for uu in range(3):
                        fetch_ws(uu)
                wm = wm_tiles.pop(u)
                pm = pp.tile([P, BL], FP32, tag="psm")
                for k in range(KT):
                    mm(pm[:], wm[:, k * P:(k + 1) * P], x_sb[:, k, :],
                       start=(k == 0), stop=(k == KT - 1))
                tm = mp.tile([P, BL], FP32, tag=f"tm{u}")
                nc.scalar.add(tm[:], pm[:], bmu_t[:, u:u + 1])
                t_m.append(tm)

            # eps_in deferred past phase 1's critical HBM window (z8/zb
            # are first consumed ~35 us later).
            for c in range(NCH):
                s = slice(c * KC, (c + 1) * KC)
                ei_c = acts.tile([P, KC, BL], BF16, tag=f"ei{c}",
                                 name=f"ei{c}")
                nc.scalar.dma_start(ei_c[:], eiT[:, s, :])
                for kk in range(KC):
                    k = c * KC + kk
                    if k < K8:
                        nc.vector.tensor_mul(z8[:, k, :], x_sb[:, k, :],
                                             ei_c[:, kk, :])
                    else:
                        nc.vector.tensor_mul(zb[:, k - K8, :], x_sb[:, k, :],
                                             ei_c[:, kk, :])

            # eps_out on gpsimd, behind the phase-1 odd-u W_mu fetches.
            for c in range(NCH):
                s = slice(c * KC, (c + 1) * KC)
                nc.gpsimd.dma_start(eo_sb[:, s, :], eoT[:, s, :])

            # ---- Phase 2: noise terms + combine. d<1024 fp8 DR, rest bf16 ----
            for u in range(UT):
                un = u + 3
                if 3 <= un < UT:
                    fetch_ws(un)
                ws8, wsb = ws_tiles.pop(u)
                last = (u >= UT - 2)
                halves = (0, BL // 2, BL) if last else (0, BL)
                for h in range(len(halves) - 1):
                    lo, hi = halves[h], halves[h + 1]
                    pn = ppn.tile([P, hi - lo], FP32, tag="psn")
                    for k2 in range(K8 // 2):
                        mm(pn[:], ws8[:, 2 * k2:2 * k2 + 2, :],
                           z8[:, 2 * k2:2 * k2 + 2, lo:hi],
                           start=(k2 == 0), stop=False, perf_mode=DR)
                    for k in range(K8):
                        mm(pn[:], wsb[:, k * P:(k + 1) * P], zb[:, k, lo:hi],
                           start=False, stop=(k == K8 - 1))
                    t_n = tp.tile([P, hi - lo], FP32, tag="tn")
                    nc.scalar.activation(t_n[:], pn[:], IDENT,
                                         bias=bsg_t[:, u:u + 1], scale=DESCALE)
                    pr = tp.tile([P, hi - lo], FP32, tag="pr")
                    nc.vector.tensor_mul(pr[:], t_n[:], eo_sb[:, u, lo:hi])
                    o = op.tile([P, hi - lo], FP32, tag="o")
                    nc.vector.tensor_add(o[:], pr[:], t_m[u][:, lo:hi])
                    if last:
                        oq = nc.sync if h == 0 else nc.scalar
                    else:
                        oq = nc.gpsimd
                    oq.dma_start(outT[u][:, lo:hi], o[:])

    nc.compile()
    return nc


def _get_nc():
    global _cached
    if _cached is None:
        _cached = _build()
    return _cached


def kernel(x, weight_mu, weight_sigma, bias_mu, bias_sigma, eps_in, eps_out,
           _trace=False):
    nc = _get_nc()

    # Host-side layout prep (transposes + casts + quantization scaling only).
    def to_pkb(a):  # [B, D] -> per-core [P, KT, BL]
        a = np.ascontiguousarray(a.astype(_NBF))
        return [
            np.ascontiguousarray(
                a[c * BL:(c + 1) * BL].T.reshape(KT, P, BL).transpose(1, 0, 2))
            for c in range(N_CORES)
        ]

    xs = to_pkb(x)
    eis = to_pkb(eps_in * S_Z)
    eos = to_pkb(eps_out)  # same transform, u in place of k

    def w_blocks(w, scale, dt):  # [D', U] -> [UT, P, (D'/128)*P]
        kt = w.shape[0] // P
        wb = (w * scale).astype(dt).reshape(kt, P, UT, P).transpose(2, 1, 0, 3)
        return np.ascontiguousarray(wb.reshape(UT, P, kt * P))

    wmu_h = w_blocks(weight_mu, 1.0, _NBF)
    wg8_h = w_blocks(weight_sigma[:D // 2], S_W, _NF8)
    wgb_h = w_blocks(weight_sigma[D // 2:], S_W, _NBF)
    bmu_h = np.ascontiguousarray(bias_mu.astype(np.float32).reshape(UT, P).T)
    bsg_h = np.ascontiguousarray(bias_sigma.astype(np.float32).reshape(UT, P).T)

    in_maps = [
        {
            "xT": xs[c],
            "eiT": eis[c],
            "eoT": eos[c],
            "wmu": wmu_h,
            "wg8": wg8_h,
            "wgb": wgb_h,
            "bmu": bmu_h,
            "bsg": bsg_h,
        }
        for c in range(N_CORES)
    ]

    res = run_bass_kernel_spmd(nc, in_maps, core_ids=list(range(N_CORES)),
                               trace=_trace)
    kernel.last_result = res

    out = np.empty((B, U), dtype=np.float32)
    for c in range(N_CORES):
        oc = res.results[c]["outT"]  # [UT, P, BL]
        out[c * BL:(c + 1) * BL] = oc.transpose(2, 0, 1).reshape(BL, U)
    return out


# revision 5
# speedup vs baseline: 1.0211x; 1.0000x over previous
"""NoisyNet dense layer on 8 TRN2 NeuronCores — baseline structure +
hidden weight loads + fp8 DoubleRow on half the noise contraction.

out[b,u] = x @ W_mu + eps_out * ((x*eps_in) @ W_sigma) + bias_mu + bias_sigma*eps_out

Same transposed layout and two-phase, stall-free DMA schedule as the
135.7us baseline ([D,B]/[U,B] on-device, data-parallel over batch), with
two measured upgrades:
  - every matmul is emitted as explicit LDWEIGHTS + a non-self-loading
    MATMUL (ins.ldweights=False): the PE pulls the loads ahead into the
    background weight buffer, cutting ~50ns/MM (265 -> ~216).
  - the noise matmul runs d<1024 in fp8e4 DoubleRow (K=256/instruction
    at the full 2x MAC rate) and d>=1024 in bf16. Full-fp8 would be
    rel-err 2.3e-2 (>2e-2 budget); this 50/50 split measures 1.65e-2.
    eps_in is prescaled by 4 and W_sigma by 1024 (powers of two; z*4
    fits fp8e4's +-240 range); the 1/4096 descale folds into the
    phase-2 ACT bias-add (scale operand), which costs nothing.
"""

import numpy as np
import ml_dtypes

import concourse.bacc as bacc
import concourse.mybir as mybir
import concourse.tile as tile
from concourse.bass_utils import run_bass_kernel_spmd

N_CORES = 8
B, D, U = 4096, 2048, 2048
BL = B // N_CORES          # 512 batch rows per core
P = 128                    # partitions
KT = D // P                # 16 contraction tiles
K8 = KT // 2               # 8 k-tiles in the fp8 half
UT = U // P                # 16 output tiles
KC = 4                     # k-tiles per activation DMA chunk
NCH = KT // KC             # 4 chunks
BF16 = mybir.dt.bfloat16
FP32 = mybir.dt.float32
FP8 = mybir.dt.float8e4

S_Z = 4.0
S_W = 1024.0
DESCALE = 1.0 / (S_Z * S_W)

_NBF = ml_dtypes.bfloat16
_NF8 = ml_dtypes.float8_e4m3

_cached = None


def _build():
    nc = bacc.Bacc("TRN2", target_bir_lowering=False, debug=False)

    # activations laid out [P, KT, BL]: partition p holds d = k*128+p
    xT = nc.declare_dram_parameter("xT", [P, KT, BL], BF16, isOutput=False)
    eiT = nc.declare_dram_parameter("eiT", [P, KT, BL], BF16, isOutput=False)
    eoT = nc.declare_dram_parameter("eoT", [P, UT, BL], BF16, isOutput=False)
    wmu = nc.declare_dram_parameter("wmu", [UT, P, KT * P], BF16, isOutput=False)
    wg8 = nc.declare_dram_parameter("wg8", [UT, P, K8 * P], FP8, isOutput=False)
    wgb = nc.declare_dram_parameter("wgb", [UT, P, K8 * P], BF16, isOutput=False)
    bmu = nc.declare_dram_parameter("bmu", [P, UT], FP32, isOutput=False)
    bsg = nc.declare_dram_parameter("bsg", [P, UT], FP32, isOutput=False)
    outT = nc.declare_dram_parameter("outT", [UT, P, BL], FP32, isOutput=True)

    DR = mybir.MatmulPerfMode.DoubleRow
    IDENT = mybir.ActivationFunctionType.Identity

    with tile.TileContext(nc) as tc:
        with (
            tc.tile_pool(name="acts", bufs=1) as acts,
            tc.tile_pool(name="w", bufs=7) as wp,
            tc.tile_pool(name="w8", bufs=4) as wp8,
            tc.tile_pool(name="bias", bufs=1) as bp,
            tc.tile_pool(name="psum", bufs=4, space="PSUM") as pp,
            tc.tile_pool(name="psumn", bufs=3, space="PSUM") as ppn,
            tc.tile_pool(name="mean", bufs=1) as mp,
            tc.tile_pool(name="tmp", bufs=2) as tp,
            tc.tile_pool(name="out", bufs=3) as op,
        ):
            # HAM warm-up so the first real matmuls run at full clock.
            warm_in = bp.tile([P, BL], BF16, tag="warmin")
            nc.gpsimd.memset(warm_in[:], 0.0)
            warm_ps = ppn.tile([P, BL], FP32, tag="psn")
            for _ in range(6):
                nc.tensor.matmul(warm_ps[:], warm_in[:, :P], warm_in[:])

            def mm(ps, stat, mov, start, stop, perf_mode=None):
                nc.tensor.ldweights(stat, perf_mode=perf_mode)
                m = nc.tensor.matmul(ps, stat, mov, start=start, stop=stop,
                                     perf_mode=perf_mode)
                m.ins.ldweights = False

            # Weight stream (sync queue): all W_mu first, then W_sigma
            # (fp8 tile + bf16 tile per u).
            wm_tiles = {}
            ws_tiles = {}

            def fetch_wm(u, split=False):
                t = wp.tile([P, KT * P], BF16, tag="wm")
                q = nc.sync if u % 2 == 0 else nc.gpsimd
                # early tiles arrive as halves so each u-block can start
                # its k<8 matmuls ~2us before the full tile lands
                if split:
                    q.dma_start(t[:, :KC * P], wmu[u][:, :KC * P])
                    q.dma_start(t[:, KC * P:2 * KC * P],
                                wmu[u][:, KC * P:2 * KC * P])
                    q.dma_start(t[:, 2 * KC * P:], wmu[u][:, 2 * KC * P:])
                elif u <= 6:
                    q.dma_start(t[:, :2 * KC * P], wmu[u][:, :2 * KC * P])
                    q.dma_start(t[:, 2 * KC * P:], wmu[u][:, 2 * KC * P:])
                else:
                    q.dma_start(t[:], wmu[u])
                wm_tiles[u] = t

            def fetch_ws(u):
                t8 = wp8.tile([P, K8, P], FP8, tag="ws8")
                nc.sync.dma_start(t8[:], wg8[u])
                tb = wp.tile([P, K8 * P], BF16, tag="wsb")
                nc.scalar.dma_start(tb[:], wgb[u])
                ws_tiles[u] = (t8, tb)

            # biases first in the gpsimd stream (needed at the first
            # phase-1 drain; the odd-u W_mu stream follows them).
            bmu_t = bp.tile([P, UT], FP32, tag="bmu")
            nc.gpsimd.dma_start(bmu_t[:], bmu[:])
            bsg_t = bp.tile([P, UT], FP32, tag="bsg")
            nc.gpsimd.dma_start(bsg_t[:], bsg[:])

            fetch_wm(0, split=True)

            # Activation stream (scalar queue): x first, then eps_in
            # (z8/zb production), then eps_out.
            x_sb = acts.tile([P, KT, BL], BF16, tag="x")
            z8 = acts.tile([P, K8, BL], FP8, tag="z8")
            zb = acts.tile([P, K8, BL], BF16, tag="zb")
            eo_sb = acts.tile([P, UT, BL], BF16, tag="eo")

            nc.scalar.dma_start(x_sb[:, 0:1, :], xT[:, 0:1, :])
            nc.scalar.dma_start(x_sb[:, 1:KC, :], xT[:, 1:KC, :])
            fetch_wm(1)
            for c in range(1, NCH):
                s = slice(c * KC, (c + 1) * KC)
                nc.scalar.dma_start(x_sb[:, s, :], xT[:, s, :])
            for uu in range(2, 6):
                fetch_wm(uu)
            # ---- Phase 1: mean terms. t_m[u] = W_mu[u].T @ x + bias_mu ----
            t_m = []
            for u in range(UT):
                if u + 6 < UT:
                    fetch_wm(u + 6)
                elif u + 6 == UT:
                    for uu in range(3):
                        fetch_ws(uu)
                wm = wm_tiles.pop(u)
                pm = pp.tile([P, BL], FP32, tag="psm")
                for k in range(KT):
                    mm(pm[:], wm[:, k * P:(k + 1) * P], x_sb[:, k, :],
                       start=(k == 0), stop=(k == KT - 1))
                tm = mp.tile([P, BL], FP32, tag=f"tm{u}")
                nc.scalar.add(tm[:], pm[:], bmu_t[:, u:u + 1])
                t_m.append(tm)

            # eps_in deferred past phase 1's critical HBM window (z8/zb
            # are first consumed ~35 us later).
            for c in range(NCH):
                s = slice(c * KC, (c + 1) * KC)
                ei_c = acts.tile([P, KC, BL], BF16, tag=f"ei{c}",
                                 name=f"ei{c}")
                nc.scalar.dma_start(ei_c[:], eiT[:, s, :])
                for kk in range(KC):
                    k = c * KC + kk
                    if k < K8:
                        nc.vector.tensor_mul(z8[:, k, :], x_sb[:, k, :],
                                             ei_c[:, kk, :])
                    else:
                        nc.vector.tensor_mul(zb[:, k - K8, :], x_sb[:, k, :],
                                             ei_c[:, kk, :])

            # eps_out on gpsimd, behind the phase-1 odd-u W_mu fetches.
            for c in range(NCH):
                s = slice(c * KC, (c + 1) * KC)
                nc.gpsimd.dma_start(eo_sb[:, s, :], eoT[:, s, :])

            # ---- Phase 2: noise terms + combine. d<1024 fp8 DR, rest bf16 ----
            for u in range(UT):
                un = u + 3
                if 3 <= un < UT:
                    fetch_ws(un)
                ws8, wsb = ws_tiles.pop(u)
                last = (u >= UT - 2)
                halves = (0, BL // 2, BL) if last else (0, BL)
                for h in range(len(halves) - 1):
                    lo, hi = halves[h], halves[h + 1]
                    pn = ppn.tile([P, hi - lo], FP32, tag="psn")
                    for k2 in range(K8 // 2):
                        mm(pn[:], ws8[:, 2 * k2:2 * k2 + 2, :],
                           z8[:, 2 * k2:2 * k2 + 2, lo:hi],
                           start=(k2 == 0), stop=False, perf_mode=DR)
                    for k in range(K8):
                        mm(pn[:], wsb[:, k * P:(k + 1) * P], zb[:, k, lo:hi],
                           start=False, stop=(k == K8 - 1))
                    t_n = tp.tile([P, hi - lo], FP32, tag="tn")
                    nc.scalar.activation(t_n[:], pn[:], IDENT,
                                         bias=bsg_t[:, u:u + 1], scale=DESCALE)
                    pr = tp.tile([P, hi - lo], FP32, tag="pr")
                    nc.vector.tensor_mul(pr[:], t_n[:], eo_sb[:, u, lo:hi])
                    o = op.tile([P, hi - lo], FP32, tag="o")
                    nc.vector.tensor_add(o[:], pr[:], t_m[u][:, lo:hi])
                    if last:
                        oq = nc.sync if h == 0 else nc.scalar
                    else:
                        oq = nc.gpsimd
                    oq.dma_start(outT[u][:, lo:hi], o[:])

    nc.compile()
    return nc


def _get_nc():
    global _cached
    if _cached is None:
        _cached = _build()
    return _cached


def kernel(x, weight_mu, weight_sigma, bias_mu, bias_sigma, eps_in, eps_out,
           _trace=False):
    nc = _get_nc()

    # Host-side layout prep (transposes + casts + quantization scaling only).
    def to_pkb(a):  # [B, D] -> per-core [P, KT, BL]
        a = np.ascontiguousarray(a.astype(_NBF))
        return [
            np.ascontiguousarray(
                a[c * BL:(c + 1) * BL].T.reshape(KT, P, BL).transpose(1, 0, 2))
            for c in range(N_CORES)
        ]

    xs = to_pkb(x)
    eis = to_pkb(eps_in * S_Z)
    eos = to_pkb(eps_out)  # same transform, u in place of k

    def w_blocks(w, scale, dt):  # [D', U] -> [UT, P, (D'/128)*P]
        kt = w.shape[0] // P
        wb = (w * scale).astype(dt).reshape(kt, P, UT, P).transpose(2, 1, 0, 3)
        return np.ascontiguousarray(wb.reshape(UT, P, kt * P))

    wmu_h = w_blocks(weight_mu, 1.0, _NBF)
    wg8_h = w_blocks(weight_sigma[:D // 2], S_W, _NF8)
    wgb_h = w_blocks(weight_sigma[D // 2:], S_W, _NBF)
    bmu_h = np.ascontiguousarray(bias_mu.astype(np.float32).reshape(UT, P).T)
    bsg_h = np.ascontiguousarray(bias_sigma.astype(np.float32).reshape(UT, P).T)

    in_maps = [
        {
            "xT": xs[c],
            "eiT": eis[c],
            "eoT": eos[c],
            "wmu": wmu_h,
            "wg8": wg8_h,
            "wgb": wgb_h,
            "bmu": bmu_h,
            "bsg": bsg_h,
        }
        for c in range(N_CORES)
    ]

    res = run_bass_kernel_spmd(nc, in_maps, core_ids=list(range(N_CORES)),
                               trace=_trace)
    kernel.last_result = res

    out = np.empty((B, U), dtype=np.float32)
    for c in range(N_CORES):
        oc = res.results[c]["outT"]  # [UT, P, BL]
        out[c * BL:(c + 1) * BL] = oc.transpose(2, 0, 1).reshape(BL, U)
    return out


# revision 6
# speedup vs baseline: 1.0387x; 1.0172x over previous
"""NoisyNet dense layer on 8 TRN2 NeuronCores — baseline structure +
hidden weight loads + fp8 DoubleRow on half the noise contraction.

out[b,u] = x @ W_mu + eps_out * ((x*eps_in) @ W_sigma) + bias_mu + bias_sigma*eps_out

Same transposed layout and two-phase, stall-free DMA schedule as the
135.7us baseline ([D,B]/[U,B] on-device, data-parallel over batch), with
two measured upgrades:
  - every matmul is emitted as explicit LDWEIGHTS + a non-self-loading
    MATMUL (ins.ldweights=False): the PE pulls the loads ahead into the
    background weight buffer, cutting ~50ns/MM (265 -> ~216).
  - the noise matmul runs d<1024 in fp8e4 DoubleRow (K=256/instruction
    at the full 2x MAC rate) and d>=1024 in bf16. Full-fp8 would be
    rel-err 2.3e-2 (>2e-2 budget); this 50/50 split measures 1.65e-2.
    eps_in is prescaled by 4 and W_sigma by 1024 (powers of two; z*4
    fits fp8e4's +-240 range); the 1/4096 descale folds into the
    phase-2 ACT bias-add (scale operand), which costs nothing.
"""

import numpy as np
import ml_dtypes

import concourse.bacc as bacc
import concourse.mybir as mybir
import concourse.tile as tile
from concourse.bass_utils import run_bass_kernel_spmd

N_CORES = 8
B, D, U = 4096, 2048, 2048
BL = B // N_CORES          # 512 batch rows per core
P = 128                    # partitions
KT = D // P                # 16 contraction tiles
KF = 10                    # fp8 k-tiles (10/16: rel err 1.85e-2 < 2e-2)
KB = KT - KF               # bf16 k-tiles
UT = U // P                # 16 output tiles
KC = 4                     # k-tiles per activation DMA chunk
NCH = KT // KC             # 4 chunks
BF16 = mybir.dt.bfloat16
FP32 = mybir.dt.float32
FP8 = mybir.dt.float8e4

S_Z = 4.0
S_W = 1024.0
DESCALE = 1.0 / (S_Z * S_W)

_NBF = ml_dtypes.bfloat16
_NF8 = ml_dtypes.float8_e4m3

_cached = None


def _build():
    nc = bacc.Bacc("TRN2", target_bir_lowering=False, debug=False)

    # activations laid out [P, KT, BL]: partition p holds d = k*128+p
    xT = nc.declare_dram_parameter("xT", [P, KT, BL], BF16, isOutput=False)
    eiT = nc.declare_dram_parameter("eiT", [P, KT, BL], BF16, isOutput=False)
    eoT = nc.declare_dram_parameter("eoT", [P, UT, BL], BF16, isOutput=False)
    wmu = nc.declare_dram_parameter("wmu", [UT, P, KT * P], BF16, isOutput=False)
    wg8 = nc.declare_dram_parameter("wg8", [UT, P, KF * P], FP8, isOutput=False)
    wgb = nc.declare_dram_parameter("wgb", [UT, P, KB * P], BF16, isOutput=False)
    bmu = nc.declare_dram_parameter("bmu", [P, UT], FP32, isOutput=False)
    bsg = nc.declare_dram_parameter("bsg", [P, UT], FP32, isOutput=False)
    outT = nc.declare_dram_parameter("outT", [UT, P, BL], FP32, isOutput=True)

    DR = mybir.MatmulPerfMode.DoubleRow
    IDENT = mybir.ActivationFunctionType.Identity

    with tile.TileContext(nc) as tc:
        with (
            tc.tile_pool(name="acts", bufs=1) as acts,
            tc.tile_pool(name="w", bufs=7) as wp,
            tc.tile_pool(name="w8", bufs=4) as wp8,
            tc.tile_pool(name="bias", bufs=1) as bp,
            tc.tile_pool(name="psum", bufs=4, space="PSUM") as pp,
            tc.tile_pool(name="psumn", bufs=3, space="PSUM") as ppn,
            tc.tile_pool(name="mean", bufs=1) as mp,
            tc.tile_pool(name="tmp", bufs=2) as tp,
            tc.tile_pool(name="out", bufs=3) as op,
        ):
            # HAM warm-up so the first real matmuls run at full clock.
            warm_in = bp.tile([P, BL], BF16, tag="warmin")
            nc.gpsimd.memset(warm_in[:], 0.0)
            warm_ps = ppn.tile([P, BL], FP32, tag="psn")
            for _ in range(6):
                nc.tensor.matmul(warm_ps[:], warm_in[:, :P], warm_in[:])

            def mm(ps, stat, mov, start, stop, perf_mode=None):
                nc.tensor.ldweights(stat, perf_mode=perf_mode)
                m = nc.tensor.matmul(ps, stat, mov, start=start, stop=stop,
                                     perf_mode=perf_mode)
                m.ins.ldweights = False

            # Weight stream (sync queue): all W_mu first, then W_sigma
            # (fp8 tile + bf16 tile per u).
            wm_tiles = {}
            ws_tiles = {}

            def fetch_wm(u, split=False):
                t = wp.tile([P, KT * P], BF16, tag="wm")
                q = nc.sync if u % 2 == 0 else nc.gpsimd
                # early tiles arrive as halves so each u-block can start
                # its k<8 matmuls ~2us before the full tile lands
                if split:
                    q.dma_start(t[:, :KC * P], wmu[u][:, :KC * P])
                    q.dma_start(t[:, KC * P:2 * KC * P],
                                wmu[u][:, KC * P:2 * KC * P])
                    q.dma_start(t[:, 2 * KC * P:], wmu[u][:, 2 * KC * P:])
                elif u <= 6:
                    q.dma_start(t[:, :2 * KC * P], wmu[u][:, :2 * KC * P])
                    q.dma_start(t[:, 2 * KC * P:], wmu[u][:, 2 * KC * P:])
                else:
                    q.dma_start(t[:], wmu[u])
                wm_tiles[u] = t

            def fetch_ws(u):
                t8 = wp8.tile([P, KF, P], FP8, tag="ws8")
                nc.sync.dma_start(t8[:], wg8[u])
                tb = wp.tile([P, KB * P], BF16, tag="wsb")
                nc.scalar.dma_start(tb[:], wgb[u])
                ws_tiles[u] = (t8, tb)

            # biases first in the gpsimd stream (needed at the first
            # phase-1 drain; the odd-u W_mu stream follows them).
            bmu_t = bp.tile([P, UT], FP32, tag="bmu")
            nc.gpsimd.dma_start(bmu_t[:], bmu[:])
            bsg_t = bp.tile([P, UT], FP32, tag="bsg")
            nc.gpsimd.dma_start(bsg_t[:], bsg[:])

            fetch_wm(0, split=True)

            # Activation stream (scalar queue): x first, then eps_in
            # (z8/zb production), then eps_out.
            x_sb = acts.tile([P, KT, BL], BF16, tag="x")
            z8 = acts.tile([P, KF, BL], FP8, tag="z8")
            zb = acts.tile([P, KB, BL], BF16, tag="zb")
            eo_sb = acts.tile([P, UT, BL], BF16, tag="eo")

            nc.scalar.dma_start(x_sb[:, 0:1, :], xT[:, 0:1, :])
            nc.scalar.dma_start(x_sb[:, 1:KC, :], xT[:, 1:KC, :])
            fetch_wm(1)
            for c in range(1, NCH):
                s = slice(c * KC, (c + 1) * KC)
                nc.scalar.dma_start(x_sb[:, s, :], xT[:, s, :])
            for uu in range(2, 6):
                fetch_wm(uu)
            # ---- Phase 1: mean terms. t_m[u] = W_mu[u].T @ x + bias_mu ----
            t_m = []
            for u in range(UT):
                if u + 6 < UT:
                    fetch_wm(u + 6)
                elif u + 6 == UT:
                    for uu in range(3):
                        fetch_ws(uu)
                wm = wm_tiles.pop(u)
                pm = pp.tile([P, BL], FP32, tag="psm")
                for k in range(KT):
                    mm(pm[:], wm[:, k * P:(k + 1) * P], x_sb[:, k, :],
                       start=(k == 0), stop=(k == KT - 1))
                tm = mp.tile([P, BL], FP32, tag=f"tm{u}")
                nc.scalar.add(tm[:], pm[:], bmu_t[:, u:u + 1])
                t_m.append(tm)

            # eps_in deferred past phase 1's critical HBM window (z8/zb
            # are first consumed ~35 us later).
            for c in range(NCH):
                s = slice(c * KC, (c + 1) * KC)
                ei_c = acts.tile([P, KC, BL], BF16, tag=f"ei{c}",
                                 name=f"ei{c}")
                nc.scalar.dma_start(ei_c[:], eiT[:, s, :])
                for kk in range(KC):
                    k = c * KC + kk
                    if k < KF:
                        nc.vector.tensor_mul(z8[:, k, :], x_sb[:, k, :],
                                             ei_c[:, kk, :])
                    else:
                        nc.vector.tensor_mul(zb[:, k - KF, :], x_sb[:, k, :],
                                             ei_c[:, kk, :])

            # eps_out on gpsimd, behind the phase-1 odd-u W_mu fetches.
            for c in range(NCH):
                s = slice(c * KC, (c + 1) * KC)
                nc.gpsimd.dma_start(eo_sb[:, s, :], eoT[:, s, :])

            # ---- Phase 2: noise terms + combine. d<1024 fp8 DR, rest bf16 ----
            for u in range(UT):
                un = u + 3
                if 3 <= un < UT:
                    fetch_ws(un)
                ws8, wsb = ws_tiles.pop(u)
                last = (u >= UT - 2)
                halves = (0, BL // 2, BL) if last else (0, BL)
                for h in range(len(halves) - 1):
                    lo, hi = halves[h], halves[h + 1]
                    pn = ppn.tile([P, hi - lo], FP32, tag="psn")
                    for k2 in range(KF // 2):
                        mm(pn[:], ws8[:, 2 * k2:2 * k2 + 2, :],
                           z8[:, 2 * k2:2 * k2 + 2, lo:hi],
                           start=(k2 == 0), stop=False, perf_mode=DR)
                    for k in range(KB):
                        mm(pn[:], wsb[:, k * P:(k + 1) * P], zb[:, k, lo:hi],
                           start=False, stop=(k == KB - 1))
                    t_n = tp.tile([P, hi - lo], FP32, tag="tn")
                    nc.scalar.activation(t_n[:], pn[:], IDENT,
                                         bias=bsg_t[:, u:u + 1], scale=DESCALE)
                    pr = tp.tile([P, hi - lo], FP32, tag="pr")
                    nc.vector.tensor_mul(pr[:], t_n[:], eo_sb[:, u, lo:hi])
                    o = op.tile([P, hi - lo], FP32, tag="o")
                    nc.vector.tensor_add(o[:], pr[:], t_m[u][:, lo:hi])
                    if last:
                        oq = nc.sync if h == 0 else nc.scalar
                    else:
                        oq = nc.gpsimd
                    oq.dma_start(outT[u][:, lo:hi], o[:])

    nc.compile()
    return nc


def _get_nc():
    global _cached
    if _cached is None:
        _cached = _build()
    return _cached


def kernel(x, weight_mu, weight_sigma, bias_mu, bias_sigma, eps_in, eps_out,
           _trace=False):
    nc = _get_nc()

    # Host-side layout prep (transposes + casts + quantization scaling only).
    def to_pkb(a):  # [B, D] -> per-core [P, KT, BL]
        a = np.ascontiguousarray(a.astype(_NBF))
        return [
            np.ascontiguousarray(
                a[c * BL:(c + 1) * BL].T.reshape(KT, P, BL).transpose(1, 0, 2))
            for c in range(N_CORES)
        ]

    xs = to_pkb(x)
    eis = to_pkb(eps_in * S_Z)
    eos = to_pkb(eps_out)  # same transform, u in place of k

    def w_blocks(w, scale, dt):  # [D', U] -> [UT, P, (D'/128)*P]
        kt = w.shape[0] // P
        wb = (w * scale).astype(dt).reshape(kt, P, UT, P).transpose(2, 1, 0, 3)
        return np.ascontiguousarray(wb.reshape(UT, P, kt * P))

    wmu_h = w_blocks(weight_mu, 1.0, _NBF)
    wg8_h = w_blocks(weight_sigma[:KF * P], S_W, _NF8)
    wgb_h = w_blocks(weight_sigma[KF * P:], S_W, _NBF)
    bmu_h = np.ascontiguousarray(bias_mu.astype(np.float32).reshape(UT, P).T)
    bsg_h = np.ascontiguousarray(bias_sigma.astype(np.float32).reshape(UT, P).T)

    in_maps = [
        {
            "xT": xs[c],
            "eiT": eis[c],
            "eoT": eos[c],
            "wmu": wmu_h,
            "wg8": wg8_h,
            "wgb": wgb_h,
            "bmu": bmu_h,
            "bsg": bsg_h,
        }
        for c in range(N_CORES)
    ]

    res = run_bass_kernel_spmd(nc, in_maps, core_ids=list(range(N_CORES)),
                               trace=_trace)
    kernel.last_result = res

    out = np.empty((B, U), dtype=np.float32)
    for c in range(N_CORES):
        oc = res.results[c]["outT"]  # [UT, P, BL]
        out[c * BL:(c + 1) * BL] = oc.transpose(2, 0, 1).reshape(BL, U)
    return out


# revision 7
# speedup vs baseline: 1.0449x; 1.0060x over previous
"""NoisyNet dense layer on 8 TRN2 NeuronCores — baseline structure +
hidden weight loads + fp8 DoubleRow on half the noise contraction.

out[b,u] = x @ W_mu + eps_out * ((x*eps_in) @ W_sigma) + bias_mu + bias_sigma*eps_out

Same transposed layout and two-phase, stall-free DMA schedule as the
135.7us baseline ([D,B]/[U,B] on-device, data-parallel over batch), with
two measured upgrades:
  - every matmul is emitted as explicit LDWEIGHTS + a non-self-loading
    MATMUL (ins.ldweights=False): the PE pulls the loads ahead into the
    background weight buffer, cutting ~50ns/MM (265 -> ~216).
  - the noise matmul runs d<1280 (10 of 16 k-tiles) in fp8e4 DoubleRow
    (K=256/instruction at the full 2x MAC rate) and the rest in bf16.
    Full-fp8 would be rel-err 2.3e-2 (>2e-2 budget); this 10/16 split
    measures a deterministic 1.832e-2 (8.4% gate margin). eps_in is
    prescaled by 4 and W_sigma by 1024 (powers of two; z*4 fits fp8e4's
    +-240 range); the 1/4096 descale folds into the phase-2 ACT
    bias-add (scale operand), which costs nothing.
  - weights ride three queues in consumption order (even-u W_mu + fp8
    W_sigma on sync; odd-u W_mu + eps_out + outputs on gpsimd; x +
    deferred eps_in + bf16 W_sigma on scalar); the first seven W_mu
    tiles arrive as half-transfers so the early u-blocks start sooner.
"""

import numpy as np
import ml_dtypes

import concourse.bacc as bacc
import concourse.mybir as mybir
import concourse.tile as tile
from concourse.bass_utils import run_bass_kernel_spmd

N_CORES = 8
B, D, U = 4096, 2048, 2048
BL = B // N_CORES          # 512 batch rows per core
P = 128                    # partitions
KT = D // P                # 16 contraction tiles
KF = 10                    # fp8 k-tiles (10/16: rel err 1.85e-2 < 2e-2)
KB = KT - KF               # bf16 k-tiles
UT = U // P                # 16 output tiles
KC = 4                     # k-tiles per activation DMA chunk
NCH = KT // KC             # 4 chunks
BF16 = mybir.dt.bfloat16
FP32 = mybir.dt.float32
FP8 = mybir.dt.float8e4

S_Z = 4.0
S_W = 1024.0
DESCALE = 1.0 / (S_Z * S_W)

_NBF = ml_dtypes.bfloat16
_NF8 = ml_dtypes.float8_e4m3

_cached = None


def _build():
    nc = bacc.Bacc("TRN2", target_bir_lowering=False, debug=False)

    # activations laid out [P, KT, BL]: partition p holds d = k*128+p
    xT = nc.declare_dram_parameter("xT", [P, KT, BL], BF16, isOutput=False)
    eiT = nc.declare_dram_parameter("eiT", [P, KT, BL], BF16, isOutput=False)
    eoT = nc.declare_dram_parameter("eoT", [P, UT, BL], BF16, isOutput=False)
    wmu = nc.declare_dram_parameter("wmu", [UT, P, KT * P], BF16, isOutput=False)
    wg8 = nc.declare_dram_parameter("wg8", [UT, P, KF * P], FP8, isOutput=False)
    wgb = nc.declare_dram_parameter("wgb", [UT, P, KB * P], BF16, isOutput=False)
    bmu = nc.declare_dram_parameter("bmu", [P, UT], FP32, isOutput=False)
    bsg = nc.declare_dram_parameter("bsg", [P, UT], FP32, isOutput=False)
    outT = nc.declare_dram_parameter("outT", [UT, P, BL], FP32, isOutput=True)

    DR = mybir.MatmulPerfMode.DoubleRow
    IDENT = mybir.ActivationFunctionType.Identity

    with tile.TileContext(nc) as tc:
        with (
            tc.tile_pool(name="acts", bufs=1) as acts,
            tc.tile_pool(name="w", bufs=7) as wp,
            tc.tile_pool(name="w8", bufs=4) as wp8,
            tc.tile_pool(name="bias", bufs=1) as bp,
            tc.tile_pool(name="psum", bufs=4, space="PSUM") as pp,
            tc.tile_pool(name="psumn", bufs=3, space="PSUM") as ppn,
            tc.tile_pool(name="mean", bufs=1) as mp,
            tc.tile_pool(name="tmp", bufs=2) as tp,
            tc.tile_pool(name="out", bufs=3) as op,
        ):
            # HAM warm-up so the first real matmuls run at full clock.
            warm_in = bp.tile([P, BL], BF16, tag="warmin")
            nc.gpsimd.memset(warm_in[:], 0.0)
            warm_ps = ppn.tile([P, BL], FP32, tag="psn")
            for _ in range(6):
                nc.tensor.matmul(warm_ps[:], warm_in[:, :P], warm_in[:])

            def mm(ps, stat, mov, start, stop, perf_mode=None):
                nc.tensor.ldweights(stat, perf_mode=perf_mode)
                m = nc.tensor.matmul(ps, stat, mov, start=start, stop=stop,
                                     perf_mode=perf_mode)
                m.ins.ldweights = False

            # Weight stream (sync queue): all W_mu first, then W_sigma
            # (fp8 tile + bf16 tile per u).
            wm_tiles = {}
            ws_tiles = {}

            def fetch_wm(u, split=False):
                t = wp.tile([P, KT * P], BF16, tag="wm")
                q = nc.sync if u % 2 == 0 else nc.gpsimd
                # early tiles arrive as halves so each u-block can start
                # its k<8 matmuls ~2us before the full tile lands
                if split:
                    q.dma_start(t[:, :KC * P], wmu[u][:, :KC * P])
                    q.dma_start(t[:, KC * P:2 * KC * P],
                                wmu[u][:, KC * P:2 * KC * P])
                    q.dma_start(t[:, 2 * KC * P:], wmu[u][:, 2 * KC * P:])
                elif u <= 6:
                    q.dma_start(t[:, :2 * KC * P], wmu[u][:, :2 * KC * P])
                    q.dma_start(t[:, 2 * KC * P:], wmu[u][:, 2 * KC * P:])
                else:
                    q.dma_start(t[:], wmu[u])
                wm_tiles[u] = t

            def fetch_ws(u):
                t8 = wp8.tile([P, KF, P], FP8, tag="ws8")
                nc.sync.dma_start(t8[:], wg8[u])
                tb = wp.tile([P, KB * P], BF16, tag="wsb")
                nc.scalar.dma_start(tb[:], wgb[u])
                ws_tiles[u] = (t8, tb)

            # biases first in the gpsimd stream (needed at the first
            # phase-1 drain; the odd-u W_mu stream follows them).
            bmu_t = bp.tile([P, UT], FP32, tag="bmu")
            nc.gpsimd.dma_start(bmu_t[:], bmu[:])
            bsg_t = bp.tile([P, UT], FP32, tag="bsg")
            nc.gpsimd.dma_start(bsg_t[:], bsg[:])

            fetch_wm(0, split=True)

            # Activation stream (scalar queue): x first, then eps_in
            # (z8/zb production), then eps_out.
            x_sb = acts.tile([P, KT, BL], BF16, tag="x")
            z8 = acts.tile([P, KF, BL], FP8, tag="z8")
            zb = acts.tile([P, KB, BL], BF16, tag="zb")
            eo_sb = acts.tile([P, UT, BL], BF16, tag="eo")

            nc.scalar.dma_start(x_sb[:, 0:1, :], xT[:, 0:1, :])
            nc.scalar.dma_start(x_sb[:, 1:KC, :], xT[:, 1:KC, :])
            fetch_wm(1)
            for c in range(1, NCH):
                s = slice(c * KC, (c + 1) * KC)
                nc.scalar.dma_start(x_sb[:, s, :], xT[:, s, :])
            for uu in range(2, 6):
                fetch_wm(uu)
            # ---- Phase 1: mean terms. t_m[u] = W_mu[u].T @ x + bias_mu ----
            t_m = []
            for u in range(UT):
                if u + 6 < UT:
                    fetch_wm(u + 6)
                elif u + 6 == UT:
                    for uu in range(3):
                        fetch_ws(uu)
                wm = wm_tiles.pop(u)
                pm = pp.tile([P, BL], FP32, tag="psm")
                for k in range(KT):
                    mm(pm[:], wm[:, k * P:(k + 1) * P], x_sb[:, k, :],
                       start=(k == 0), stop=(k == KT - 1))
                tm = mp.tile([P, BL], FP32, tag=f"tm{u}")
                nc.scalar.add(tm[:], pm[:], bmu_t[:, u:u + 1])
                t_m.append(tm)

            # eps_in deferred past phase 1's critical HBM window (z8/zb
            # are first consumed ~35 us later).
            for c in range(NCH):
                s = slice(c * KC, (c + 1) * KC)
                ei_c = acts.tile([P, KC, BL], BF16, tag=f"ei{c}",
                                 name=f"ei{c}")
                nc.scalar.dma_start(ei_c[:], eiT[:, s, :])
                for kk in range(KC):
                    k = c * KC + kk
                    if k < KF:
                        nc.vector.tensor_mul(z8[:, k, :], x_sb[:, k, :],
                                             ei_c[:, kk, :])
                    else:
                        nc.vector.tensor_mul(zb[:, k - KF, :], x_sb[:, k, :],
                                             ei_c[:, kk, :])

            # eps_out on gpsimd, behind the phase-1 odd-u W_mu fetches.
            for c in range(NCH):
                s = slice(c * KC, (c + 1) * KC)
                nc.gpsimd.dma_start(eo_sb[:, s, :], eoT[:, s, :])

            # ---- Phase 2: noise terms + combine. d<1024 fp8 DR, rest bf16 ----
            for u in range(UT):
                un = u + 3
                if 3 <= un < UT:
                    fetch_ws(un)
                ws8, wsb = ws_tiles.pop(u)
                last = (u >= UT - 2)
                halves = (0, BL // 2, BL) if last else (0, BL)
                for h in range(len(halves) - 1):
                    lo, hi = halves[h], halves[h + 1]
                    pn = ppn.tile([P, hi - lo], FP32, tag="psn")
                    for k2 in range(KF // 2):
                        mm(pn[:], ws8[:, 2 * k2:2 * k2 + 2, :],
                           z8[:, 2 * k2:2 * k2 + 2, lo:hi],
                           start=(k2 == 0), stop=False, perf_mode=DR)
                    for k in range(KB):
                        mm(pn[:], wsb[:, k * P:(k + 1) * P], zb[:, k, lo:hi],
                           start=False, stop=(k == KB - 1))
                    t_n = tp.tile([P, hi - lo], FP32, tag="tn")
                    nc.scalar.activation(t_n[:], pn[:], IDENT,
                                         bias=bsg_t[:, u:u + 1], scale=DESCALE)
                    pr = tp.tile([P, hi - lo], FP32, tag="pr")
                    nc.vector.tensor_mul(pr[:], t_n[:], eo_sb[:, u, lo:hi])
                    o = op.tile([P, hi - lo], FP32, tag="o")
                    nc.vector.tensor_add(o[:], pr[:], t_m[u][:, lo:hi])
                    if last:
                        oq = nc.sync if h == 0 else nc.scalar
                    else:
                        oq = nc.gpsimd
                    oq.dma_start(outT[u][:, lo:hi], o[:])

    nc.compile()
    return nc


def _get_nc():
    global _cached
    if _cached is None:
        _cached = _build()
    return _cached


def kernel(x, weight_mu, weight_sigma, bias_mu, bias_sigma, eps_in, eps_out,
           _trace=False):
    nc = _get_nc()

    # Host-side layout prep (transposes + casts + quantization scaling only).
    def to_pkb(a):  # [B, D] -> per-core [P, KT, BL]
        a = np.ascontiguousarray(a.astype(_NBF))
        return [
            np.ascontiguousarray(
                a[c * BL:(c + 1) * BL].T.reshape(KT, P, BL).transpose(1, 0, 2))
            for c in range(N_CORES)
        ]

    xs = to_pkb(x)
    eis = to_pkb(eps_in * S_Z)
    eos = to_pkb(eps_out)  # same transform, u in place of k

    def w_blocks(w, scale, dt):  # [D', U] -> [UT, P, (D'/128)*P]
        kt = w.shape[0] // P
        wb = (w * scale).astype(dt).reshape(kt, P, UT, P).transpose(2, 1, 0, 3)
        return np.ascontiguousarray(wb.reshape(UT, P, kt * P))

    wmu_h = w_blocks(weight_mu, 1.0, _NBF)
    wg8_h = w_blocks(weight_sigma[:KF * P], S_W, _NF8)
    wgb_h = w_blocks(weight_sigma[KF * P:], S_W, _NBF)
    bmu_h = np.ascontiguousarray(bias_mu.astype(np.float32).reshape(UT, P).T)
    bsg_h = np.ascontiguousarray(bias_sigma.astype(np.float32).reshape(UT, P).T)

    in_maps = [
        {
            "xT": xs[c],
            "eiT": eis[c],
            "eoT": eos[c],
            "wmu": wmu_h,
            "wg8": wg8_h,
            "wgb": wgb_h,
            "bmu": bmu_h,
            "bsg": bsg_h,
        }
        for c in range(N_CORES)
    ]

    res = run_bass_kernel_spmd(nc, in_maps, core_ids=list(range(N_CORES)),
                               trace=_trace)
    kernel.last_result = res

    out = np.empty((B, U), dtype=np.float32)
    for c in range(N_CORES):
        oc = res.results[c]["outT"]  # [UT, P, BL]
        out[c * BL:(c + 1) * BL] = oc.transpose(2, 0, 1).reshape(BL, U)
    return out
